# revision 7
# baseline (speedup 1.0000x reference)
"""Trainium2 Bass kernel for one-hop GNN mean aggregation + per-clip projection.

Computation (see reference):
    nodes [2048, 10] int64  -> flat n = 20480 node ids in [0, 50000)
    adj   [50000, 32] int64 -> neighbor lists
    features [50000, 256] f32
    local_weight [8, 128, 256] f32
    out[n, c, k] = relu( mean_j features[adj[nodes[n], j]] @ local_weight[c, k, :] )
    returned as [2048, 10, 8, 128] f32

Strategy (v3):
  - node dedup: only the ~16880 unique node ids are computed on device
    (padded to 8*17*128 = 17408, 2176 per core, 17 chunks of 128); the host
    expands rows back to the 20480 flat slots afterwards.
  - the 81920-row irregular gather per core is done with the custom
    InstDMAGatherAnt ucode (dma_gather) in TRANSPOSE mode: rows land
    feature-dim-on-partitions so the neighbor sum is a unit-stride
    tensor_reduce producing the matmul operand directly (no transposes).
  - int16 index limit handled by a two-pass split table (zero rows at 0 and
    50001, pass HI rebased at row 17234); per-node padding to the chunk max
    per side hits a zero row; nodes sorted per core by low-count so the
    padding overhead is a few percent.
  - Q7 descriptor generation is the serial bottleneck, so gathers are
    merged: chunk PAIRS share one gather instruction per side, with
    dynamic_dma_scratch_size=65536 for descriptor-ring headroom.
  - index lists are [16, N/16]-wrapped and replicated on all 128 partitions
    (the Q7 core pair reads its own 16-partition group).
  - single_packet=False (required beyond ~512 indices).

Per chunk: two tensor_reduce (lo/hi windows) -> f32 [128d, 2, 128n],
add+cast to bf16 featT, accumulating bf16 matmul against host-pretransposed
W^T, fused (1/32 scale + ReLU) on ACT out of PSUM, f32 rows DMA'd out.
"""

import numpy as np
from ml_dtypes import bfloat16

import concourse.bass as bass
import concourse.bacc as bacc
import concourse.mybir as mybir
import concourse.tile as tile
from concourse.bass_utils import run_bass_kernel_spmd

N_CORES = 8
NUM_NODES = 50000
FEAT_DIM = 256
CLIPS = 8
DIM = 128
K_NEIGH = 32
B, S = 2048, 10
N_FLAT = B * S                      # 20480
P = 128
N_CHUNKS = 17
N_PER_CORE = N_CHUNKS * P           # 2176
N_UNIQ_PAD = N_CORES * N_PER_CORE   # 17408
CK = CLIPS * DIM                    # 1024

NUM_DEV = NUM_NODES + 2             # device table: [zero, features, zero]
LO_MAX = 32766                      # ids <= LO_MAX -> pass LO (local = id+1)
HI_BASE = 17234                     # pass HI base row; local = id+1-HI_BASE
                                    # dummy 32767 -> row 50001 (zero)

GROUPS = [[0, 1], [2, 3], [4, 5], [6, 7], [8, 9], [10, 11], [12, 13], [14, 15], [16]]

_last_results = None  # BassKernelResults of the most recent run (for test.py)


def build_program(w_lists):
    """w_lists: per-chunk (W_lo, W_hi), identical across cores."""
    nc = bacc.Bacc(
        "TRN2",
        target_bir_lowering=False,
        debug=False,
        num_devices=N_CORES,
        dynamic_dma_scratch_size=65536,
    )
    s_total = sum(8 * (wl + wh) for wl, wh in w_lists)
    feat_d = nc.dram_tensor("features", [NUM_DEV, FEAT_DIM], mybir.dt.bfloat16, kind="ExternalInput")
    idx_d = nc.dram_tensor("idxs", [P, s_total], mybir.dt.int16, kind="ExternalInput")
    w_d = nc.dram_tensor("w_t", [2, P, CK], mybir.dt.bfloat16, kind="ExternalInput")
    out_d = nc.dram_tensor("out", [N_PER_CORE, CK], mybir.dt.float32, kind="ExternalOutput")

    bf16 = mybir.dt.bfloat16
    f32 = mybir.dt.float32
    add = mybir.AluOpType.add

    with tile.TileContext(nc) as tc:
        with (
            tc.tile_pool(name="const", bufs=1) as const_pool,
            tc.tile_pool(name="work", bufs=2) as work,
            tc.tile_pool(name="gath", bufs=2) as gath_pool,
            tc.tile_pool(name="psum_o", bufs=2, space=bass.MemorySpace.PSUM) as psum_o,
        ):
            w_sb = const_pool.tile([P, 2, CK], bf16)
            for h in range(2):
                nc.sync.dma_start(out=w_sb[:, h, :], in_=w_d[h, :, :])

            idx_sb = const_pool.tile([P, s_total], mybir.dt.int16)
            nc.sync.dma_start(out=idx_sb[:], in_=idx_d[:, :])

            s_off = 0

            def gather(widths, base):
                """One dma_gather covering len(widths) chunks on one side."""
                nonlocal s_off
                n_idx = P * sum(widths)
                if n_idx == 0:
                    return None
                s_len = n_idx // 16
                g = gath_pool.tile([P, 2, n_idx], bf16)
                nc.gpsimd.dma_gather(
                    g[:],
                    feat_d[base : base + 32768, :],
                    idx_sb[:, s_off : s_off + s_len],
                    n_idx,
                    n_idx,
                    FEAT_DIM,
                    transpose=True,
                    single_packet=False,
                )
                s_off += s_len
                return g

            for grp in GROUPS:
                lo_ws = [w_lists[ch][0] for ch in grp]
                hi_ws = [w_lists[ch][1] for ch in grp]
                g_lo = gather(lo_ws, 0)
                g_hi = gather(hi_ws, HI_BASE)

                lo_off = 0
                hi_off = 0
                for k, ch in enumerate(grp):
                    w_lo, w_hi = w_lists[ch]
                    reds = []
                    for g, w_side, off in ((g_lo, w_lo, lo_off), (g_hi, w_hi, hi_off)):
                        if w_side == 0 or g is None:
                            continue
                        red = work.tile([P, 2, P], f32)
                        nc.vector.tensor_reduce(
                            out=red[:],
                            in_=g[:, :, off : off + P * w_side].rearrange(
                                "p h (n w) -> p h n w", w=w_side
                            ),
                            axis=mybir.AxisListType.X,
                            op=add,
                        )
                        reds.append(red)
                    lo_off += P * w_lo
                    hi_off += P * w_hi

                    featT = work.tile([P, 2, P], bf16)
                    if len(reds) == 2:
                        nc.vector.tensor_tensor(
                            out=featT[:], in0=reds[0][:], in1=reds[1][:], op=add
                        )
                    else:
                        nc.vector.tensor_copy(out=featT[:], in_=reds[0][:])

                    po = psum_o.tile([P, CK], f32)
                    for nb in range(2):
                        cols = slice(nb * 512, (nb + 1) * 512)
                        for h in range(2):
                            nc.tensor.matmul(
                                po[:, cols],
                                featT[:, h, :],
                                w_sb[:, h, cols],
                                start=(h == 0),
                                stop=(h == 1),
                            )

                    out_t = work.tile([P, CK], f32)
                    nc.scalar.activation(
                        out=out_t[:],
                        in_=po[:],
                        func=mybir.ActivationFunctionType.Relu,
                        scale=1.0 / K_NEIGH,
                    )
                    nc.sync.dma_start(
                        out=out_d[ch * P : (ch + 1) * P, :], in_=out_t[:]
                    )

    nc.compile()
    return nc


def _core_widths(neigh_core):
    lo_cnt = (neigh_core <= LO_MAX).sum(axis=1)
    order = np.argsort(lo_cnt, kind="stable")
    w_lists = []
    for ch in range(N_CHUNKS):
        lc = lo_cnt[order[ch * P : (ch + 1) * P]]
        w_lists.append((int(lc.max()), int((K_NEIGH - lc).max())))
    return w_lists


def _build_blocks(neigh_core, w_unified):
    """Index blocks for one core, emitted in the kernel's consumption order:
    per group, lo blocks for all chunks then hi blocks for all chunks."""
    lo_mask = neigh_core <= LO_MAX
    lo_cnt = lo_mask.sum(axis=1)
    order = np.argsort(lo_cnt, kind="stable")
    blocks = []

    def side_block(chunks, is_lo):
        cols = []
        for ch in chunks:
            w_side = w_unified[ch][0 if is_lo else 1]
            if w_side == 0:
                continue
            sel = order[ch * P : (ch + 1) * P]
            nb = neigh_core[sel]
            lm = lo_mask[sel]
            arr = (
                np.zeros((P, w_side), dtype=np.int16)
                if is_lo
                else np.full((P, w_side), 32767, dtype=np.int16)
            )
            for p in range(P):
                vals = nb[p][lm[p]] + 1 if is_lo else nb[p][~lm[p]] + 1 - HI_BASE
                arr[p, : len(vals)] = vals.astype(np.int16)
            cols.append(arr.reshape(-1))
        if not cols:
            return None
        flat = np.concatenate(cols)
        n_idx = len(flat)
        blk = np.zeros((16, n_idx // 16), dtype=np.int16)
        m = np.arange(n_idx)
        blk[m % 16, m // 16] = flat
        return blk

    for grp in GROUPS:
        for is_lo in (True, False):
            blk = side_block(grp, is_lo)
            if blk is not None:
                blocks.append(blk)
    return order, blocks


def prep(nodes, adj, features, local_weight):
    nodes_flat = np.asarray(nodes).reshape(-1)
    adj_np = np.asarray(adj)
    uniq, inverse = np.unique(nodes_flat, return_inverse=True)
    uniq_pad = np.full(N_UNIQ_PAD, uniq[0], dtype=uniq.dtype)
    uniq_pad[: len(uniq)] = uniq
    neigh = adj_np[uniq_pad]                          # [N_UNIQ_PAD, K]

    feat = np.asarray(features).astype(bfloat16)
    feat_dev = np.zeros((NUM_DEV, FEAT_DIM), dtype=bfloat16)
    feat_dev[1 : NUM_NODES + 1] = feat
    w = np.asarray(local_weight).astype(np.float32)
    w_t = np.ascontiguousarray(
        w.transpose(2, 0, 1).reshape(2, P, CK).astype(bfloat16)
    )

    cores_neigh = [
        neigh[c * N_PER_CORE : (c + 1) * N_PER_CORE] for c in range(N_CORES)
    ]
    w_lists_all = [_core_widths(nb) for nb in cores_neigh]
    w_unified = [
        (
            max(wl[ch][0] for wl in w_lists_all),
            max(wl[ch][1] for wl in w_lists_all),
        )
        for ch in range(N_CHUNKS)
    ]

    in_maps = []
    orders = []
    for c in range(N_CORES):
        order, blocks = _build_blocks(cores_neigh[c], w_unified)
        orders.append(order)
        idx_arr = np.concatenate(blocks, axis=1)
        idx_full = np.tile(idx_arr, (8, 1))
        in_maps.append(
            {
                "features": feat_dev,
                "idxs": np.ascontiguousarray(idx_full),
                "w_t": w_t,
            }
        )
    return w_unified, in_maps, orders, inverse


_program_cache = {}


def kernel(nodes, adj, features, local_weight, trace=False):
    global _last_results
    w_unified, in_maps, orders, inverse = prep(nodes, adj, features, local_weight)
    key = tuple(w_unified)
    if key not in _program_cache:
        _program_cache.clear()
        _program_cache[key] = build_program(w_unified)
    nc = _program_cache[key]
    res = run_bass_kernel_spmd(
        nc, in_maps, core_ids=list(range(N_CORES)), trace=trace
    )
    _last_results = res

    # unique-node results in original (unsorted) per-core order
    uniq_rows = np.empty((N_UNIQ_PAD, CK), dtype=np.float32)
    for c in range(N_CORES):
        dev_rows = res.results[c]["out"]              # sorted order
        pos = np.empty(N_PER_CORE, dtype=np.int64)
        pos[orders[c]] = np.arange(N_PER_CORE)
        uniq_rows[c * N_PER_CORE : (c + 1) * N_PER_CORE] = dev_rows[pos]
    out = uniq_rows[inverse]                          # expand to flat slots
    return out.reshape(B, S, CLIPS, DIM)
